# revision 31
# baseline (speedup 1.0000x reference)
"""Multi-head attention (B=16, N=577, C=768, H=12) on 8 TRN2 NeuronCores.

Strategy: pure data parallelism over batch (2 images per core, no
collectives). Per core, everything is computed "channels-on-partitions"
(transposed) so that no on-device transposes are ever needed:

  qkT[outc, tok]  = qkv_wT-tiles.T @ xT          (q scaled 1/8 + bias on evict)
  V[tok, outc]    = xT-tiles.T @ qkv_wT          (natural layout, + bias;
                                                  col 0 of each 65-block = 1)
  S^T[nk, nq]     = K^T-tiles.T @ Q^T            (K=64 contraction)
  E^T             = exp(S^T) * exp(relbT)        (host precomputes exp of the
                                                  transposed rel-pos bias; no
                                                  max subtraction -- logits are
                                                  bounded ~|7| for this problem)
  O'^T[65, nq]    = [1 | V_h]-tiles.T @ E^T      (row 0 = softmax denominator)
  O^T             = O'^T[1:65] * bcast(1/O'^T[0])
  out^T[co, tok]  = projT-tiles.T @ O^T + proj_b

Performance structure (~220 us -> this version targets ~150 us):
  - software-pipelined attention: the S matmuls + exps of pair k+1 are
    emitted BEFORE the O' (AV) matmuls of pair k, so the PE never waits on
    the exp/rel-bias chain of the pair it is about to reduce
  - per-pair S/exp tiles are pair-merged (staP/rbaP hold both heads) so the
    rel-bias multiply is 2 wide DVE ops per pair instead of 4
  - denominator row is FIRST in the O' psum (ones col 0 of V) so
    reciprocal_approx_fast (reads physical partition 0) needs no staging copy
  - engine rebalance: ACT keeps only exps + bias-evicts that need it
    (Q evict -> DVE tensor_scalar, K evict -> GpSimd copy, V bias ->
    GpSimd, O' psoB evict -> GpSimd, b0-proj 512-evicts -> GpSimd)
  - startup: V-block weight columns + x are DMA'd before the Q/K weight
    columns so the V projection starts ~8 us earlier
  - tail: two b0-proj sets are held back to fill the PE while the last
    pair's normalize chain completes; b1 proj (all-ACT evicts) follows
  - heads processed in pairs (rows 0:64 / 64:128) so consecutive LDWEIGHTS
    alternate PE row groups and can overlap in-flight matmuls

Host side pre-transposes all inputs (and converts to bf16) and transposes
the output back. PSUM accumulation is f32 throughout.
"""
import numpy as np
import ml_dtypes

B, N, C, H, HD = 16, 577, 768, 12, 64
NCORES = 8
BPC = B // NCORES          # batches per core: 2
NT = BPC * N               # tokens per core: 1154
P = 128

# token-free-dim chunks over NT (matmul free dim <= 512 for f32 psum)
TFREE = [(0, 512), (512, 512), (1024, 130)]
# nk (key token) tiles over N
NKT = [(0, 128), (128, 128), (256, 128), (384, 128), (512, 65)]
# nq (query token) chunks over N
NQF = [(0, 512), (512, 65)]

_CACHE = {}


def _build():
    import concourse.tile as tile
    from concourse import bacc, mybir

    bf16 = mybir.dt.bfloat16
    f32 = mybir.dt.float32
    Alu = mybir.AluOpType
    Act = mybir.ActivationFunctionType

    nc = bacc.Bacc(
        "TRN2",
        target_bir_lowering=False,
        debug=False,
        enable_asserts=False,
        num_devices=NCORES,
    )
    xT = nc.dram_tensor("xT", [C, NT], bf16, kind="ExternalInput").ap()
    wqkvT = nc.dram_tensor("wqkvT", [C, 3 * C], bf16, kind="ExternalInput").ap()
    qbias = nc.dram_tensor("qbias", [P, 6], f32, kind="ExternalInput").ap()
    vbias = nc.dram_tensor("vbias", [1, C], f32, kind="ExternalInput").ap()
    relbT = nc.dram_tensor("relbT", [H, 640, N], bf16, kind="ExternalInput").ap()
    projT = nc.dram_tensor("projT", [C, C], bf16, kind="ExternalInput").ap()
    pbias = nc.dram_tensor("pbias", [P, 6], f32, kind="ExternalInput").ap()
    out = nc.dram_tensor("out", [C, NT], f32, kind="ExternalOutput").ap()

    with tile.TileContext(nc) as tc:
        with (
            tc.tile_pool(name="persist", bufs=1) as pp,
            tc.tile_pool(name="relb", bufs=2) as relp,
            tc.tile_pool(name="st", bufs=2) as stp,
            tc.tile_pool(name="dn", bufs=3) as dnp,
            tc.tile_pool(name="oev", bufs=3) as oevp,
            tc.tile_pool(name="pss", bufs=2, space="PSUM") as ps_s,
            tc.tile_pool(name="psrump", bufs=1, space="PSUM") as ps_r,
            tc.tile_pool(name="pso", bufs=2, space="PSUM") as ps_o,
        ):
            # ---------------- Phase A: load weights / constants ----------
            # tiny bias DMAs first, then the V-block weight columns + x (the
            # V projection only needs those), then the Q/K weight columns.
            qb = pp.tile([P, 6], f32, tag="qb", name="qb")
            nc.sync.dma_start(qb[:], qbias[:])
            pb = pp.tile([P, 6], f32, tag="pb", name="pb")
            nc.sync.dma_start(pb[:], pbias[:])
            vbr = pp.tile([1, C], f32, tag="vbr", name="vbr")
            nc.sync.dma_start(vbr[:], vbias[:])
            vb = pp.tile([P, C], f32, tag="vb", name="vb")
            nc.gpsimd.partition_broadcast(vb[:, :], vbr[0:1, :])
            xtall = pp.tile([P, 6, NT], bf16, tag="xtall", name="xtall")
            wtall = pp.tile([P, 6, 3 * C], bf16, tag="wtall", name="wtall")
            ptall = pp.tile([P, 6, C], bf16, tag="ptall", name="ptall")
            xt = [xtall[:, i, :] for i in range(6)]
            wt = [wtall[:, i, :] for i in range(6)]
            pt = [ptall[:, i, :] for i in range(6)]
            # one strided DMA each: V-block weight columns first (the V
            # projection only needs those + x), then x, then Q/K columns
            nc.sync.dma_start(
                wtall[:, :, 2 * C : 3 * C],
                wqkvT[:, 2 * C : 3 * C].rearrange("(i p) n -> p i n", p=P),
            )
            nc.sync.dma_start(
                xtall[:, :, :], xT[:, :].rearrange("(i p) n -> p i n", p=P)
            )
            nc.sync.dma_start(
                wtall[:, :, 0 : 2 * C],
                wqkvT[:, 0 : 2 * C].rearrange("(i p) n -> p i n", p=P),
            )

            # ---------------- persistent result tiles ----------------------
            # qk[t] for t in 0..11: [128, NT] bf16, outc block t (q: 0-5, k: 6-11)
            qk = []
            for t in range(12):
                qk.append(pp.tile([P, NT], bf16, tag=f"qk{t}", name=f"qk{t}"))
            # o[t]: [128, NT] bf16 -- O^T assembled for the projection
            o = []
            for t in range(6):
                o.append(pp.tile([P, NT], bf16, tag=f"o{t}", name=f"o{t}"))
            v = [[None] * 5 for _ in range(BPC)]

            def qk_group(t):
                # Q^T/K^T projection for outc block t; Q evicts on DVE
                # (tensor_scalar mul+bias), K evicts on GpSimd (plain copy)
                for (f0, fsz) in TFREE:
                    ps = ps_s.tile([P, 1024], f32, tag="ps_s", name="psmm")
                    for ki in range(6):
                        nc.tensor.matmul(
                            ps[:, 0:fsz],
                            wt[ki][:, P * t : P * (t + 1)],
                            xt[ki][:, f0 : f0 + fsz],
                            start=(ki == 0),
                            stop=(ki == 5),
                        )
                    if t < 6:  # q: scale 1/8 + bias (pre-scaled on host)
                        nc.scalar.activation(
                            qk[t][:, f0 : f0 + fsz],
                            ps[:, 0:fsz],
                            Act.Identity,
                            bias=qb[:, t : t + 1],
                            scale=0.125,
                        )
                    else:  # k: plain copy (k bias is zero)
                        nc.vector.tensor_copy(qk[t][:, f0 : f0 + fsz], ps[:, 0:fsz])

            def v_group(b, j):
                # V projection (natural layout) for batch b, token tile j
                # v[b][j]: [nksz, 780] bf16, 12 head-blocks of [1 | V_h(64)]
                nk0, nksz = NKT[j]
                vt = pp.tile([P, 12 * 65], bf16, tag=f"v{b}_{j}", name=f"v{b}_{j}")
                v[b][j] = vt
                v3 = vt[:, :].rearrange("p (h w) -> p h w", w=65)
                nc.gpsimd.memset(v3[:, :, 64:65], 1.0)
                tok0 = b * N + nk0
                for half in range(2):  # outc halves of 384 = 6 heads
                    f0 = 384 * half
                    ps = ps_s.tile([P, 1024], f32, tag="ps_s", name="psmm")
                    for ki in range(6):
                        nc.tensor.matmul(
                            ps[0:nksz, 0:384],
                            xt[ki][:, tok0 : tok0 + nksz],
                            wt[ki][:, 2 * C + f0 : 2 * C + f0 + 384],
                            start=(ki == 0),
                            stop=(ki == 5),
                        )
                    ps3 = ps[0:nksz, 0:384].rearrange("p (h w) -> p h w", w=64)
                    vb3 = vb[0:nksz, f0 : f0 + 384].rearrange(
                        "p (h w) -> p h w", w=64
                    )
                    nc.vector.tensor_tensor(
                        v3[0:nksz, 6 * half : 6 * half + 6, 0:64],
                        ps3[:, :, :],
                        vb3[:, :, :],
                        op=Alu.add,
                    )

            def proj_group(t, f0, fsz, eng):
                ps = ps_s.tile([P, 1024], f32, tag="ps_s", name="psmm")
                for ki in range(6):
                    nc.tensor.matmul(
                        ps[:, 0:fsz],
                        pt[ki][:, P * t : P * (t + 1)],
                        o[ki][:, f0 : f0 + fsz],
                        start=(ki == 0),
                        stop=(ki == 5),
                    )
                ot = oevp.tile([P, 512], f32, tag="oev", name="oev")
                if eng == "act":
                    nc.scalar.activation(
                        ot[:, 0:fsz], ps[:, 0:fsz], Act.Identity,
                        bias=pb[:, t : t + 1],
                    )
                else:
                    nc.vector.tensor_scalar(
                        ot[:, 0:fsz], ps[:, 0:fsz], pb[:, t : t + 1], None,
                        op0=Alu.add,
                    )
                nc.sync.dma_start(out[P * t : P * (t + 1), f0 : f0 + fsz], ot[:, 0:fsz])

            # -------------- pipelined attention phases ---------------------
            # staP/rbaP: [128, 2*5N] bf16, head parity pr at cols [pr*5N, (pr+1)*5N)
            def s_prologue(b, h0):
                # rel-bias DMAs + tile allocs for the pair (h0, h0+1)
                rbaP = relp.tile([P, 10 * N], bf16, tag="rba", name="rba")
                staP = stp.tile([P, 10 * N], bf16, tag="sta", name="sta")
                for hh in (h0, h0 + 1):
                    pr = hh % 2
                    r3 = rbaP[:, 5 * N * pr : 5 * N * pr + 5 * N].rearrange(
                        "p (j q) -> p j q", q=N
                    )
                    nc.sync.dma_start(
                        r3[:, :, :],
                        relbT[hh, 0:640, :].rearrange("(j p) q -> p j q", p=P),
                    )
                rump = ps_r.tile([P, 1024], f32, tag="rump", name="rump")
                return staP, rbaP, rump

            def s_step(b, h0, st, j):
                # S matmuls + one pair-merged exp evict for nk tile j
                staP, rbaP, rump = st
                qt = h0 // 2
                nk0, nksz = NKT[j]
                ps = ps_s.tile([P, 1024], f32, tag="ps_s", name="pss")
                for hh in (h0, h0 + 1):
                    pr = hh % 2
                    qoff = pr * 64
                    lk = qk[6 + qt][qoff : qoff + 64, b * N + nk0 : b * N + nk0 + nksz]
                    nc.tensor.matmul(
                        ps[0:nksz, 512 * pr : 512 * pr + 512],
                        lk,
                        qk[qt][qoff : qoff + 64, b * N : b * N + 512],
                        start=True,
                        stop=True,
                    )
                    nc.tensor.matmul(
                        rump[0:nksz, 512 * pr + 65 * j : 512 * pr + 65 * j + 65],
                        lk,
                        qk[qt][qoff : qoff + 64, b * N + 512 : b * N + N],
                        start=True,
                        stop=True,
                    )
                # one exp for both heads' 512-chunks (adjacent psum banks)
                s2 = staP[:, :].rearrange("p (h q) -> p h q", h=2)
                p2 = ps[:, :].rearrange("p (h q) -> p h q", h=2)
                nc.scalar.activation(
                    s2[0:nksz, :, N * j : N * j + 512],
                    p2[0:nksz, :, :],
                    Act.Exp,
                )

            def s_epilogue(st):
                # one strided exp for all ten 65-wide rumps of the pair
                # (rows 65:128 of the j=4 chunks hold garbage -- never read)
                staP, rbaP, rump = st
                s3 = staP[:, :].rearrange("p (h j q) -> p h j q", h=2, q=N)
                r3 = rump[:, :].rearrange("p (h q) -> p h q", h=2)[
                    :, :, 0:325
                ].rearrange("p h (j q) -> p h j q", q=65)
                nc.scalar.activation(
                    s3[:, :, 0:5, 512:577], r3[:, :, :, :], Act.Exp
                )

            def mult_phase(st):
                # multiplicative rel-bias, both heads at once; first 3 nk
                # blocks on DVE, last 2 on GpSimd (engine balance)
                staP, rbaP, rump = st
                s2 = staP[:, :].rearrange("p (h q) -> p h q", h=2)
                r2 = rbaP[:, :].rearrange("p (h q) -> p h q", h=2)
                nc.vector.tensor_tensor(
                    s2[:, :, 0 : 3 * N], s2[:, :, 0 : 3 * N], r2[:, :, 0 : 3 * N],
                    op=Alu.mult,
                )
                nc.vector.tensor_tensor(
                    s2[:, :, 3 * N : 5 * N], s2[:, :, 3 * N : 5 * N],
                    r2[:, :, 3 * N : 5 * N],
                    op=Alu.mult,
                )

            def av_alloc(hh):
                # pass-1 psum: [65, 512] (one bank per head, both heads live)
                ost = dnp.tile([64, N], bf16, tag="ost", name="ost")
                ps1 = ps_o.tile([65, 512], f32, tag="o", name="pso1")
                return ost, ps1

            def av_step(b, hh, av, staP, j):
                ost, ps1 = av
                pr = hh % 2
                nk0, nksz = NKT[j]
                lv = v[b][j][0:nksz, 65 * hh : 65 * hh + 65]
                nc.tensor.matmul(
                    ps1[0:65, 0:512],
                    lv,
                    staP[0:nksz, 5 * N * pr + N * j : 5 * N * pr + N * j + 512],
                    start=(j == 0),
                    stop=(j == 4),
                )

            def av_evict1(av):
                # denominator row straight out of PSUM (parallel with the
                # O eviction): dr on ACT (small), O rows on DVE
                ost, ps1 = av
                dr = dnp.tile([1, N], f32, tag="dr", name="dr")
                nc.vector.tensor_copy(dr[0:1, 0:512], ps1[64:65, 0:512])
                nc.vector.tensor_copy(ost[0:64, 0:512], ps1[0:64, 0:512])
                return dr

            def av_pass2(b, h0, avs, drs, staP):
                # nq rump (cols 512:577) for both heads, one ps_r-pool tile
                # (bank-split h0/h1); then finish: evict + recip + broadcast
                ps2 = ps_r.tile([P, 1024], f32, tag="rump", name="pso2")
                fins = {}
                for hh in (h0, h0 + 1):
                    pr = hh % 2
                    for j, (nk0, nksz) in enumerate(NKT):
                        lv = v[b][j][0:nksz, 65 * hh : 65 * hh + 65]
                        nc.tensor.matmul(
                            ps2[0:65, 512 * pr : 512 * pr + 65],
                            lv,
                            staP[0:nksz, 5 * N * pr + N * j + 512 : 5 * N * pr + N * j + N],
                            start=(j == 0),
                            stop=(j == 4),
                        )
                for hh in (h0, h0 + 1):
                    pr = hh % 2
                    ost = avs[hh][0]
                    dr = drs[hh]
                    nc.vector.tensor_copy(dr[0:1, 512:577], ps2[64:65, 512 * pr : 512 * pr + 65])
                    nc.vector.tensor_copy(
                        ost[0:64, 512:577], ps2[0:64, 512 * pr : 512 * pr + 65]
                    )
                    rr = dnp.tile([1, N], f32, tag="rr", name="rr")
                    nc.vector.reciprocal_approx_fast(rr[0:1, 0:N], dr[0:1, 0:N])
                    rb = dnp.tile([64, N], f32, tag="rbb", name="rbb")
                    nc.gpsimd.partition_broadcast(rb[0:64, 0:N], rr[0:1, 0:N])
                    fins[hh] = (ost, rb)
                return fins

            def omult(b, hh, fin):
                ost, rb = fin
                qt = hh // 2
                qoff = (hh % 2) * 64
                nc.vector.tensor_tensor(
                    o[qt][qoff : qoff + 64, b * N : b * N + N],
                    ost[0:64, 0:N],
                    rb[0:64, 0:N],
                    op=Alu.mult,
                )

            # proj token chunks, batch-aligned
            PFREE0 = [(0, 512), (512, 65)]           # batch 0 tokens
            PFREE1 = [(577, 512), (1089, 65)]        # batch 1 tokens

            # ------------- pipelined emission schedule ---------------------
            for j in range(5):
                v_group(0, j)
            qk_group(0)
            qk_group(6)
            qk_group(1)
            qk_group(7)

            pairs = [(b, h0) for b in range(BPC) for h0 in range(0, 12, 2)]
            # lead-in: pair 0's S runs without an AV partner
            st = s_prologue(*pairs[0])
            for j in range(5):
                s_step(*pairs[0], st, j)
            s_epilogue(st)
            mult_phase(st)
            cur = {0: st}
            for k in range(12):
                b, h0 = pairs[k]
                staP = cur[k][0]
                avs = {hh: av_alloc(hh) for hh in (h0, h0 + 1)}
                if k + 1 < 12:
                    nb, nh0 = pairs[k + 1]
                    cur[k + 1] = s_prologue(nb, nh0)
                    # interleave: S of pair k+1 with AV of pair k, per nk tile
                    for j in range(5):
                        s_step(nb, nh0, cur[k + 1], j)
                        for hh in (h0, h0 + 1):
                            av_step(b, hh, avs[hh], staP, j)
                    s_epilogue(cur[k + 1])
                else:
                    for j in range(5):
                        for hh in (h0, h0 + 1):
                            av_step(b, hh, avs[hh], staP, j)
                drs = {hh: av_evict1(avs[hh]) for hh in (h0, h0 + 1)}
                if k + 1 < 12:
                    mult_phase(cur[k + 1])
                fins = av_pass2(b, h0, avs, drs, staP)
                for hh in (h0, h0 + 1):
                    omult(b, hh, fins[hh])
                del cur[k]
                # fillers: dense PE work with no dependence on pair k+1
                if k <= 3:                       # Q/K projections, 2 pairs ahead
                    qk_group(k + 2)
                    qk_group(k + 8)
                elif k == 4:
                    nc.sync.dma_start(
                        ptall[:, :, :], projT[:, :].rearrange("(i p) n -> p i n", p=P)
                    )
                    v_group(1, 0)
                    v_group(1, 1)
                elif k == 5:
                    v_group(1, 2)
                    v_group(1, 3)
                    v_group(1, 4)
                elif 7 <= k <= 10:               # b0 proj, one outc tile each
                    t = k - 7
                    for (f0, fsz) in PFREE0:
                        proj_group(t, f0, fsz, "act")
            # tail: held-back b0 proj fills the last normalize wait
            for t in (4, 5):
                for (f0, fsz) in PFREE0:
                    proj_group(t, f0, fsz, "act")
            # ---------------- remaining output projection ------------------
            for t in range(6):
                proj_group(t, PFREE1[0][0], PFREE1[0][1], "act")
            for t in range(6):
                proj_group(t, PFREE1[1][0], PFREE1[1][1], "act")

    nc.compile()
    return nc


def _get_nc():
    if "nc" not in _CACHE:
        _CACHE["nc"] = _build()
    return _CACHE["nc"]


def make_in_maps(x, rel_pos_bias, qkv_w, q_bias, v_bias, proj_w, proj_b):
    bf = ml_dtypes.bfloat16
    x = np.asarray(x, dtype=np.float32)
    rel_pos_bias = np.asarray(rel_pos_bias, dtype=np.float32)
    qkv_w = np.asarray(qkv_w, dtype=np.float32)
    q_bias = np.asarray(q_bias, dtype=np.float32)
    v_bias = np.asarray(v_bias, dtype=np.float32)
    proj_w = np.asarray(proj_w, dtype=np.float32)
    proj_b = np.asarray(proj_b, dtype=np.float32)

    wqkvT = np.ascontiguousarray(qkv_w.T).astype(bf)                    # [768, 2304]
    qbias = np.ascontiguousarray((q_bias * 0.125).reshape(6, P).T)      # [128, 6]
    vbias = np.ascontiguousarray(v_bias[None, :])                       # [1, 768]
    # exp of the transposed rel-pos bias: applied multiplicatively after exp(S);
    # rows padded 577->640 (5*128) so each head loads as a single strided DMA
    relbT = np.zeros((H, 640, N), dtype=bf)
    relbT[:, :N, :] = np.exp(rel_pos_bias[0].transpose(0, 2, 1)).astype(bf)
    projT = np.ascontiguousarray(proj_w.T).astype(bf)                   # [768, 768]
    pbias = np.ascontiguousarray(proj_b.reshape(6, P).T)                # [128, 6]

    in_maps = []
    for c in range(NCORES):
        xT = np.ascontiguousarray(
            x[BPC * c : BPC * (c + 1)].reshape(NT, C).T
        ).astype(bf)                                                    # [768, 1154]
        in_maps.append(
            dict(
                xT=xT,
                wqkvT=wqkvT,
                qbias=qbias,
                vbias=vbias,
                relbT=relbT,
                projT=projT,
                pbias=pbias,
            )
        )
    return in_maps


def kernel(x, rel_pos_bias, qkv_w, q_bias, v_bias, proj_w, proj_b):
    from concourse import bass_utils

    in_maps = make_in_maps(x, rel_pos_bias, qkv_w, q_bias, v_bias, proj_w, proj_b)
    nc = _get_nc()
    res = bass_utils.run_bass_kernel_spmd(nc, in_maps, core_ids=list(range(NCORES)))
    outs = []
    for c in range(NCORES):
        oT = res.results[c]["out"]                                      # [768, 1154]
        outs.append(np.ascontiguousarray(oT.T).reshape(BPC, N, C))
    return np.concatenate(outs, axis=0)


# revision 32
# speedup vs baseline: 1.2030x; 1.2030x over previous
"""Multi-head attention (B=16, N=577, C=768, H=12) on 8 TRN2 NeuronCores.

Strategy: pure data parallelism over batch (2 images per core, no
collectives). Per core, everything is computed "channels-on-partitions"
(transposed) so that no on-device transposes are ever needed:

  qkT[outc, tok]  = qkv_wT-tiles.T @ xT          (q scaled 1/8 + bias on evict)
  V[tok, outc]    = xT-tiles.T @ qkv_wT          (natural layout, + bias;
                                                  col 0 of each 65-block = 1)
  S^T[nk, nq]     = K^T-tiles.T @ Q^T            (K=64 contraction)
  E^T             = exp(S^T) * exp(relbT)        (host precomputes exp of the
                                                  transposed rel-pos bias; no
                                                  max subtraction -- logits are
                                                  bounded ~|7| for this problem)
  O'^T[65, nq]    = [1 | V_h]-tiles.T @ E^T      (row 0 = softmax denominator)
  O^T             = O'^T[1:65] * bcast(1/O'^T[0])
  out^T[co, tok]  = projT-tiles.T @ O^T + proj_b

Performance structure (~220 us -> this version targets ~150 us):
  - software-pipelined attention: the S matmuls + exps of pair k+1 are
    emitted BEFORE the O' (AV) matmuls of pair k, so the PE never waits on
    the exp/rel-bias chain of the pair it is about to reduce
  - per-pair S/exp tiles are pair-merged (staP/rbaP hold both heads) so the
    rel-bias multiply is 2 wide DVE ops per pair instead of 4
  - denominator row is FIRST in the O' psum (ones col 0 of V) so
    reciprocal_approx_fast (reads physical partition 0) needs no staging copy
  - engine rebalance: ACT keeps only exps + bias-evicts that need it
    (Q evict -> DVE tensor_scalar, K evict -> GpSimd copy, V bias ->
    GpSimd, O' psoB evict -> GpSimd, b0-proj 512-evicts -> GpSimd)
  - startup: V-block weight columns + x are DMA'd before the Q/K weight
    columns so the V projection starts ~8 us earlier
  - tail: two b0-proj sets are held back to fill the PE while the last
    pair's normalize chain completes; b1 proj (all-ACT evicts) follows
  - heads processed in pairs (rows 0:64 / 64:128) so consecutive LDWEIGHTS
    alternate PE row groups and can overlap in-flight matmuls

Host side pre-transposes all inputs (and converts to bf16) and transposes
the output back. PSUM accumulation is f32 throughout.
"""
import numpy as np
import ml_dtypes

B, N, C, H, HD = 16, 577, 768, 12, 64
NCORES = 8
BPC = B // NCORES          # batches per core: 2
NT = BPC * N               # tokens per core: 1154
P = 128

# token-free-dim chunks over NT (matmul free dim <= 512 for f32 psum)
TFREE = [(0, 512), (512, 512), (1024, 130)]
# nk (key token) tiles over N
NKT = [(0, 128), (128, 128), (256, 128), (384, 128), (512, 65)]
# nq (query token) chunks over N
NQF = [(0, 512), (512, 65)]

_CACHE = {}


def _build():
    import concourse.tile as tile
    from concourse import bacc, mybir

    bf16 = mybir.dt.bfloat16
    f32 = mybir.dt.float32
    Alu = mybir.AluOpType
    Act = mybir.ActivationFunctionType

    nc = bacc.Bacc(
        "TRN2",
        target_bir_lowering=False,
        debug=False,
        enable_asserts=False,
        num_devices=NCORES,
    )
    xT = nc.dram_tensor("xT", [C, NT], bf16, kind="ExternalInput").ap()
    wqkvT = nc.dram_tensor("wqkvT", [C, 3 * C], bf16, kind="ExternalInput").ap()
    qbias = nc.dram_tensor("qbias", [P, 6], f32, kind="ExternalInput").ap()
    vbias = nc.dram_tensor("vbias", [1, C], f32, kind="ExternalInput").ap()
    relbT = nc.dram_tensor("relbT", [H, 640, N], bf16, kind="ExternalInput").ap()
    projT = nc.dram_tensor("projT", [C, C], bf16, kind="ExternalInput").ap()
    pbias = nc.dram_tensor("pbias", [P, 6], f32, kind="ExternalInput").ap()
    out = nc.dram_tensor("out", [C, NT], f32, kind="ExternalOutput").ap()

    with tile.TileContext(nc) as tc:
        with (
            tc.tile_pool(name="persist", bufs=1) as pp,
            tc.tile_pool(name="relb", bufs=2) as relp,
            tc.tile_pool(name="st", bufs=2) as stp,
            tc.tile_pool(name="dn", bufs=3) as dnp,
            tc.tile_pool(name="oev", bufs=3) as oevp,
            tc.tile_pool(name="pss", bufs=2, space="PSUM") as ps_s,
            tc.tile_pool(name="psrump", bufs=1, space="PSUM") as ps_r,
            tc.tile_pool(name="pso", bufs=2, space="PSUM") as ps_o,
        ):
            # ---------------- Phase A: load weights / constants ----------
            # tiny bias DMAs first, then the V-block weight columns + x (the
            # V projection only needs those), then the Q/K weight columns.
            qb = pp.tile([P, 6], f32, tag="qb", name="qb")
            nc.sync.dma_start(qb[:], qbias[:])
            pb = pp.tile([P, 6], f32, tag="pb", name="pb")
            nc.sync.dma_start(pb[:], pbias[:])
            vbr = pp.tile([1, C], f32, tag="vbr", name="vbr")
            nc.sync.dma_start(vbr[:], vbias[:])
            vb = pp.tile([P, C], f32, tag="vb", name="vb")
            nc.gpsimd.partition_broadcast(vb[:, :], vbr[0:1, :])
            xtall = pp.tile([P, 6, NT], bf16, tag="xtall", name="xtall")
            wtall = pp.tile([P, 6, 3 * C], bf16, tag="wtall", name="wtall")
            ptall = pp.tile([P, 6, C], bf16, tag="ptall", name="ptall")
            xt = [xtall[:, i, :] for i in range(6)]
            wt = [wtall[:, i, :] for i in range(6)]
            pt = [ptall[:, i, :] for i in range(6)]
            # one strided DMA each: V-block weight columns first (the V
            # projection only needs those + x), then x, then Q/K columns
            nc.sync.dma_start(
                wtall[:, :, 2 * C : 3 * C],
                wqkvT[:, 2 * C : 3 * C].rearrange("(i p) n -> p i n", p=P),
            )
            nc.sync.dma_start(
                xtall[:, :, :], xT[:, :].rearrange("(i p) n -> p i n", p=P)
            )
            nc.sync.dma_start(
                wtall[:, :, 0 : 2 * C],
                wqkvT[:, 0 : 2 * C].rearrange("(i p) n -> p i n", p=P),
            )

            # ---------------- persistent result tiles ----------------------
            # qk[t] for t in 0..11: [128, NT] bf16, outc block t (q: 0-5, k: 6-11)
            qk = []
            for t in range(12):
                qk.append(pp.tile([P, NT], bf16, tag=f"qk{t}", name=f"qk{t}"))
            # o[t]: [128, NT] bf16 -- O^T assembled for the projection
            o = []
            for t in range(6):
                o.append(pp.tile([P, NT], bf16, tag=f"o{t}", name=f"o{t}"))
            v = [[None] * 5 for _ in range(BPC)]

            def qk_group(t):
                # Q^T/K^T projection for outc block t; Q evicts on DVE
                # (tensor_scalar mul+bias), K evicts on GpSimd (plain copy)
                for (f0, fsz) in TFREE:
                    ps = ps_s.tile([P, 1024], f32, tag="ps_s", name="psmm")
                    for ki in range(6):
                        nc.tensor.matmul(
                            ps[:, 0:fsz],
                            wt[ki][:, P * t : P * (t + 1)],
                            xt[ki][:, f0 : f0 + fsz],
                            start=(ki == 0),
                            stop=(ki == 5),
                        )
                    if t < 6:  # q: scale 1/8 + bias (pre-scaled on host)
                        nc.scalar.activation(
                            qk[t][:, f0 : f0 + fsz],
                            ps[:, 0:fsz],
                            Act.Identity,
                            bias=qb[:, t : t + 1],
                            scale=0.125,
                        )
                    else:  # k: plain copy (k bias is zero)
                        nc.scalar.copy(qk[t][:, f0 : f0 + fsz], ps[:, 0:fsz])

            def v_group(b, j):
                # V projection (natural layout) for batch b, token tile j
                # v[b][j]: [nksz, 780] bf16, 12 head-blocks of [1 | V_h(64)]
                nk0, nksz = NKT[j]
                vt = pp.tile([P, 12 * 65], bf16, tag=f"v{b}_{j}", name=f"v{b}_{j}")
                v[b][j] = vt
                v3 = vt[:, :].rearrange("p (h w) -> p h w", w=65)
                nc.gpsimd.memset(v3[:, :, 64:65], 1.0)
                tok0 = b * N + nk0
                for half in range(2):  # outc halves of 384 = 6 heads
                    f0 = 384 * half
                    ps = ps_s.tile([P, 1024], f32, tag="ps_s", name="psmm")
                    for ki in range(6):
                        nc.tensor.matmul(
                            ps[0:nksz, 0:384],
                            xt[ki][:, tok0 : tok0 + nksz],
                            wt[ki][:, 2 * C + f0 : 2 * C + f0 + 384],
                            start=(ki == 0),
                            stop=(ki == 5),
                        )
                    ps3 = ps[0:nksz, 0:384].rearrange("p (h w) -> p h w", w=64)
                    vb3 = vb[0:nksz, f0 : f0 + 384].rearrange(
                        "p (h w) -> p h w", w=64
                    )
                    nc.vector.tensor_tensor(
                        v3[0:nksz, 6 * half : 6 * half + 6, 0:64],
                        ps3[:, :, :],
                        vb3[:, :, :],
                        op=Alu.add,
                    )

            def proj_group(t, f0, fsz, eng):
                ps = ps_s.tile([P, 1024], f32, tag="ps_s", name="psmm")
                for ki in range(6):
                    nc.tensor.matmul(
                        ps[:, 0:fsz],
                        pt[ki][:, P * t : P * (t + 1)],
                        o[ki][:, f0 : f0 + fsz],
                        start=(ki == 0),
                        stop=(ki == 5),
                    )
                ot = oevp.tile([P, 512], f32, tag="oev", name="oev")
                if eng == "act":
                    nc.scalar.activation(
                        ot[:, 0:fsz], ps[:, 0:fsz], Act.Identity,
                        bias=pb[:, t : t + 1],
                    )
                else:
                    nc.vector.tensor_scalar(
                        ot[:, 0:fsz], ps[:, 0:fsz], pb[:, t : t + 1], None,
                        op0=Alu.add,
                    )
                nc.sync.dma_start(out[P * t : P * (t + 1), f0 : f0 + fsz], ot[:, 0:fsz])

            # -------------- pipelined attention phases ---------------------
            # staP/rbaP: [128, 2*5N] bf16, head parity pr at cols [pr*5N, (pr+1)*5N)
            def s_prologue(b, h0):
                # rel-bias DMAs + tile allocs for the pair (h0, h0+1)
                rbaP = relp.tile([P, 10 * N], bf16, tag="rba", name="rba")
                staP = stp.tile([P, 10 * N], bf16, tag="sta", name="sta")
                for hh in (h0, h0 + 1):
                    pr = hh % 2
                    r3 = rbaP[:, 5 * N * pr : 5 * N * pr + 5 * N].rearrange(
                        "p (j q) -> p j q", q=N
                    )
                    nc.sync.dma_start(
                        r3[:, :, :],
                        relbT[hh, 0:640, :].rearrange("(j p) q -> p j q", p=P),
                    )
                rump = ps_r.tile([P, 1024], f32, tag="rump", name="rump")
                return staP, rbaP, rump

            def s_step(b, h0, st, j):
                # S matmuls + one pair-merged exp evict for nk tile j
                staP, rbaP, rump = st
                qt = h0 // 2
                nk0, nksz = NKT[j]
                ps = ps_s.tile([P, 1024], f32, tag="ps_s", name="pss")
                for hh in (h0, h0 + 1):
                    pr = hh % 2
                    qoff = pr * 64
                    lk = qk[6 + qt][qoff : qoff + 64, b * N + nk0 : b * N + nk0 + nksz]
                    nc.tensor.matmul(
                        ps[0:nksz, 512 * pr : 512 * pr + 512],
                        lk,
                        qk[qt][qoff : qoff + 64, b * N : b * N + 512],
                        start=True,
                        stop=True,
                    )
                    nc.tensor.matmul(
                        rump[0:nksz, 512 * pr + 65 * j : 512 * pr + 65 * j + 65],
                        lk,
                        qk[qt][qoff : qoff + 64, b * N + 512 : b * N + N],
                        start=True,
                        stop=True,
                    )
                # one exp for both heads' 512-chunks (adjacent psum banks)
                s2 = staP[:, :].rearrange("p (h q) -> p h q", h=2)
                p2 = ps[:, :].rearrange("p (h q) -> p h q", h=2)
                nc.scalar.activation(
                    s2[0:nksz, :, N * j : N * j + 512],
                    p2[0:nksz, :, :],
                    Act.Exp,
                )

            def s_epilogue(st):
                # one strided exp for all ten 65-wide rumps of the pair
                # (rows 65:128 of the j=4 chunks hold garbage -- never read)
                staP, rbaP, rump = st
                s3 = staP[:, :].rearrange("p (h j q) -> p h j q", h=2, q=N)
                r3 = rump[:, :].rearrange("p (h q) -> p h q", h=2)[
                    :, :, 0:325
                ].rearrange("p h (j q) -> p h j q", q=65)
                nc.scalar.activation(
                    s3[:, :, 0:5, 512:577], r3[:, :, :, :], Act.Exp
                )

            def mult_phase(st):
                # multiplicative rel-bias, both heads at once; first 3 nk
                # blocks on DVE, last 2 on GpSimd (engine balance)
                staP, rbaP, rump = st
                s2 = staP[:, :].rearrange("p (h q) -> p h q", h=2)
                r2 = rbaP[:, :].rearrange("p (h q) -> p h q", h=2)
                nc.vector.tensor_tensor(
                    s2[:, :, 0 : 3 * N], s2[:, :, 0 : 3 * N], r2[:, :, 0 : 3 * N],
                    op=Alu.mult,
                )
                nc.vector.tensor_tensor(
                    s2[:, :, 3 * N : 5 * N], s2[:, :, 3 * N : 5 * N],
                    r2[:, :, 3 * N : 5 * N],
                    op=Alu.mult,
                )

            def av_alloc(hh):
                # pass-1 psum: [65, 512] (one bank per head, both heads live)
                ost = dnp.tile([64, N], bf16, tag="ost", name="ost")
                ps1 = ps_o.tile([65, 512], f32, tag="o", name="pso1")
                return ost, ps1

            def av_step(b, hh, av, staP, j):
                ost, ps1 = av
                pr = hh % 2
                nk0, nksz = NKT[j]
                lv = v[b][j][0:nksz, 65 * hh : 65 * hh + 65]
                nc.tensor.matmul(
                    ps1[0:65, 0:512],
                    lv,
                    staP[0:nksz, 5 * N * pr + N * j : 5 * N * pr + N * j + 512],
                    start=(j == 0),
                    stop=(j == 4),
                )

            def av_evict1(av):
                # denominator row straight out of PSUM (parallel with the
                # O eviction): dr on ACT (small), O rows on DVE
                ost, ps1 = av
                dr = dnp.tile([1, N], f32, tag="dr", name="dr")
                nc.vector.tensor_copy(dr[0:1, 0:512], ps1[64:65, 0:512])
                nc.vector.tensor_copy(ost[0:64, 0:512], ps1[0:64, 0:512])
                return dr

            def av_pass2(b, h0, avs, drs, staP):
                # nq rump (cols 512:577) for both heads, one ps_r-pool tile
                # (bank-split h0/h1); then finish: evict + recip + broadcast
                ps2 = ps_r.tile([P, 1024], f32, tag="rump", name="pso2")
                fins = {}
                for hh in (h0, h0 + 1):
                    pr = hh % 2
                    for j, (nk0, nksz) in enumerate(NKT):
                        lv = v[b][j][0:nksz, 65 * hh : 65 * hh + 65]
                        nc.tensor.matmul(
                            ps2[0:65, 512 * pr : 512 * pr + 65],
                            lv,
                            staP[0:nksz, 5 * N * pr + N * j + 512 : 5 * N * pr + N * j + N],
                            start=(j == 0),
                            stop=(j == 4),
                        )
                for hh in (h0, h0 + 1):
                    pr = hh % 2
                    ost = avs[hh][0]
                    dr = drs[hh]
                    nc.vector.tensor_copy(dr[0:1, 512:577], ps2[64:65, 512 * pr : 512 * pr + 65])
                    nc.vector.tensor_copy(
                        ost[0:64, 512:577], ps2[0:64, 512 * pr : 512 * pr + 65]
                    )
                    rr = dnp.tile([1, N], f32, tag="rr", name="rr")
                    nc.vector.reciprocal_approx_fast(rr[0:1, 0:N], dr[0:1, 0:N])
                    rb = dnp.tile([64, N], f32, tag="rbb", name="rbb")
                    nc.gpsimd.partition_broadcast(rb[0:64, 0:N], rr[0:1, 0:N])
                    fins[hh] = (ost, rb)
                return fins

            def omult(b, hh, fin):
                ost, rb = fin
                qt = hh // 2
                qoff = (hh % 2) * 64
                nc.vector.tensor_tensor(
                    o[qt][qoff : qoff + 64, b * N : b * N + N],
                    ost[0:64, 0:N],
                    rb[0:64, 0:N],
                    op=Alu.mult,
                )

            # proj token chunks, batch-aligned
            PFREE0 = [(0, 512), (512, 65)]           # batch 0 tokens
            PFREE1 = [(577, 512), (1089, 65)]        # batch 1 tokens

            # ------------- pipelined emission schedule ---------------------
            for j in range(5):
                v_group(0, j)
            qk_group(0)
            qk_group(6)
            qk_group(1)
            qk_group(7)

            pairs = [(b, h0) for b in range(BPC) for h0 in range(0, 12, 2)]
            # lead-in: pair 0's S runs without an AV partner
            st = s_prologue(*pairs[0])
            for j in range(5):
                s_step(*pairs[0], st, j)
            s_epilogue(st)
            mult_phase(st)
            cur = {0: st}
            for k in range(12):
                b, h0 = pairs[k]
                staP = cur[k][0]
                avs = {hh: av_alloc(hh) for hh in (h0, h0 + 1)}
                if k + 1 < 12:
                    nb, nh0 = pairs[k + 1]
                    cur[k + 1] = s_prologue(nb, nh0)
                    # interleave: S of pair k+1 with AV of pair k, per nk tile
                    for j in range(5):
                        s_step(nb, nh0, cur[k + 1], j)
                        for hh in (h0, h0 + 1):
                            av_step(b, hh, avs[hh], staP, j)
                    s_epilogue(cur[k + 1])
                else:
                    for j in range(5):
                        for hh in (h0, h0 + 1):
                            av_step(b, hh, avs[hh], staP, j)
                drs = {hh: av_evict1(avs[hh]) for hh in (h0, h0 + 1)}
                if k + 1 < 12:
                    mult_phase(cur[k + 1])
                fins = av_pass2(b, h0, avs, drs, staP)
                for hh in (h0, h0 + 1):
                    omult(b, hh, fins[hh])
                del cur[k]
                # fillers: dense PE work with no dependence on pair k+1
                if k <= 3:                       # Q/K projections, 2 pairs ahead
                    qk_group(k + 2)
                    qk_group(k + 8)
                elif k == 4:
                    nc.sync.dma_start(
                        ptall[:, :, :], projT[:, :].rearrange("(i p) n -> p i n", p=P)
                    )
                    v_group(1, 0)
                    v_group(1, 1)
                elif k == 5:
                    v_group(1, 2)
                    v_group(1, 3)
                    v_group(1, 4)
                elif 7 <= k <= 10:               # b0 proj, one outc tile each
                    t = k - 7
                    for (f0, fsz) in PFREE0:
                        proj_group(t, f0, fsz, "act")
            # tail: held-back b0 proj fills the last normalize wait
            for t in (4, 5):
                for (f0, fsz) in PFREE0:
                    proj_group(t, f0, fsz, "act")
            # ---------------- remaining output projection ------------------
            for t in range(6):
                proj_group(t, PFREE1[0][0], PFREE1[0][1], "act")
            for t in range(6):
                proj_group(t, PFREE1[1][0], PFREE1[1][1], "act")

    nc.compile()
    return nc


def _get_nc():
    if "nc" not in _CACHE:
        _CACHE["nc"] = _build()
    return _CACHE["nc"]


def make_in_maps(x, rel_pos_bias, qkv_w, q_bias, v_bias, proj_w, proj_b):
    bf = ml_dtypes.bfloat16
    x = np.asarray(x, dtype=np.float32)
    rel_pos_bias = np.asarray(rel_pos_bias, dtype=np.float32)
    qkv_w = np.asarray(qkv_w, dtype=np.float32)
    q_bias = np.asarray(q_bias, dtype=np.float32)
    v_bias = np.asarray(v_bias, dtype=np.float32)
    proj_w = np.asarray(proj_w, dtype=np.float32)
    proj_b = np.asarray(proj_b, dtype=np.float32)

    wqkvT = np.ascontiguousarray(qkv_w.T).astype(bf)                    # [768, 2304]
    qbias = np.ascontiguousarray((q_bias * 0.125).reshape(6, P).T)      # [128, 6]
    vbias = np.ascontiguousarray(v_bias[None, :])                       # [1, 768]
    # exp of the transposed rel-pos bias: applied multiplicatively after exp(S);
    # rows padded 577->640 (5*128) so each head loads as a single strided DMA
    relbT = np.zeros((H, 640, N), dtype=bf)
    relbT[:, :N, :] = np.exp(rel_pos_bias[0].transpose(0, 2, 1)).astype(bf)
    projT = np.ascontiguousarray(proj_w.T).astype(bf)                   # [768, 768]
    pbias = np.ascontiguousarray(proj_b.reshape(6, P).T)                # [128, 6]

    in_maps = []
    for c in range(NCORES):
        xT = np.ascontiguousarray(
            x[BPC * c : BPC * (c + 1)].reshape(NT, C).T
        ).astype(bf)                                                    # [768, 1154]
        in_maps.append(
            dict(
                xT=xT,
                wqkvT=wqkvT,
                qbias=qbias,
                vbias=vbias,
                relbT=relbT,
                projT=projT,
                pbias=pbias,
            )
        )
    return in_maps


def kernel(x, rel_pos_bias, qkv_w, q_bias, v_bias, proj_w, proj_b):
    from concourse import bass_utils

    in_maps = make_in_maps(x, rel_pos_bias, qkv_w, q_bias, v_bias, proj_w, proj_b)
    nc = _get_nc()
    res = bass_utils.run_bass_kernel_spmd(nc, in_maps, core_ids=list(range(NCORES)))
    outs = []
    for c in range(NCORES):
        oT = res.results[c]["out"]                                      # [768, 1154]
        outs.append(np.ascontiguousarray(oT.T).reshape(BPC, N, C))
    return np.concatenate(outs, axis=0)


# revision 34
# speedup vs baseline: 1.2070x; 1.0033x over previous
"""Multi-head attention (B=16, N=577, C=768, H=12) on 8 TRN2 NeuronCores.

Strategy: pure data parallelism over batch (2 images per core, no
collectives). Per core, everything is computed "channels-on-partitions"
(transposed) so that no on-device transposes are ever needed:

  qkT[outc, tok]  = qkv_wT-tiles.T @ xT          (q scaled 1/8 + bias on evict)
  V[tok, outc]    = xT-tiles.T @ qkv_wT          (natural layout, + bias;
                                                  col 0 of each 65-block = 1)
  S^T[nk, nq]     = K^T-tiles.T @ Q^T            (K=64 contraction)
  E^T             = exp(S^T) * exp(relbT)        (host precomputes exp of the
                                                  transposed rel-pos bias; no
                                                  max subtraction -- logits are
                                                  bounded ~|7| for this problem)
  O'^T[65, nq]    = [1 | V_h]-tiles.T @ E^T      (row 0 = softmax denominator)
  O^T             = O'^T[1:65] * bcast(1/O'^T[0])
  out^T[co, tok]  = projT-tiles.T @ O^T + proj_b

Performance structure (~220 us -> this version targets ~150 us):
  - software-pipelined attention: the S matmuls + exps of pair k+1 are
    emitted BEFORE the O' (AV) matmuls of pair k, so the PE never waits on
    the exp/rel-bias chain of the pair it is about to reduce
  - per-pair S/exp tiles are pair-merged (staP/rbaP hold both heads) so the
    rel-bias multiply is 2 wide DVE ops per pair instead of 4
  - denominator row is FIRST in the O' psum (ones col 0 of V) so
    reciprocal_approx_fast (reads physical partition 0) needs no staging copy
  - engine rebalance: ACT keeps only exps + bias-evicts that need it
    (Q evict -> DVE tensor_scalar, K evict -> GpSimd copy, V bias ->
    GpSimd, O' psoB evict -> GpSimd, b0-proj 512-evicts -> GpSimd)
  - startup: V-block weight columns + x are DMA'd before the Q/K weight
    columns so the V projection starts ~8 us earlier
  - tail: two b0-proj sets are held back to fill the PE while the last
    pair's normalize chain completes; b1 proj (all-ACT evicts) follows
  - heads processed in pairs (rows 0:64 / 64:128) so consecutive LDWEIGHTS
    alternate PE row groups and can overlap in-flight matmuls

Host side pre-transposes all inputs (and converts to bf16) and transposes
the output back. PSUM accumulation is f32 throughout.
"""
import numpy as np
import ml_dtypes

B, N, C, H, HD = 16, 577, 768, 12, 64
NCORES = 8
BPC = B // NCORES          # batches per core: 2
NT = BPC * N               # tokens per core: 1154
P = 128

# token-free-dim chunks over NT (matmul free dim <= 512 for f32 psum)
TFREE = [(0, 512), (512, 512), (1024, 130)]
# nk (key token) tiles over N
NKT = [(0, 128), (128, 128), (256, 128), (384, 128), (512, 65)]
# nq (query token) chunks over N
NQF = [(0, 512), (512, 65)]

_CACHE = {}


def _build():
    import concourse.tile as tile
    from concourse import bacc, mybir

    bf16 = mybir.dt.bfloat16
    f32 = mybir.dt.float32
    Alu = mybir.AluOpType
    Act = mybir.ActivationFunctionType

    nc = bacc.Bacc(
        "TRN2",
        target_bir_lowering=False,
        debug=False,
        enable_asserts=False,
        num_devices=NCORES,
    )
    xT = nc.dram_tensor("xT", [C, NT], bf16, kind="ExternalInput").ap()
    wqkvT = nc.dram_tensor("wqkvT", [C, 3 * C], bf16, kind="ExternalInput").ap()
    qbias = nc.dram_tensor("qbias", [P, 6], f32, kind="ExternalInput").ap()
    vbias = nc.dram_tensor("vbias", [1, C], f32, kind="ExternalInput").ap()
    relbT = nc.dram_tensor("relbT", [H, 640, N], bf16, kind="ExternalInput").ap()
    projT = nc.dram_tensor("projT", [C, C], bf16, kind="ExternalInput").ap()
    pbias = nc.dram_tensor("pbias", [P, 6], f32, kind="ExternalInput").ap()
    out = nc.dram_tensor("out", [C, NT], f32, kind="ExternalOutput").ap()

    with tile.TileContext(nc) as tc:
        with (
            tc.tile_pool(name="persist", bufs=1) as pp,
            tc.tile_pool(name="relb", bufs=2) as relp,
            tc.tile_pool(name="st", bufs=2) as stp,
            tc.tile_pool(name="dn", bufs=3) as dnp,
            tc.tile_pool(name="oev", bufs=3) as oevp,
            tc.tile_pool(name="pss", bufs=2, space="PSUM") as ps_s,
            tc.tile_pool(name="psrump", bufs=1, space="PSUM") as ps_r,
            tc.tile_pool(name="pso", bufs=2, space="PSUM") as ps_o,
        ):
            # ---------------- Phase A: load weights / constants ----------
            # tiny bias DMAs first, then the V-block weight columns + x (the
            # V projection only needs those), then the Q/K weight columns.
            qb = pp.tile([P, 6], f32, tag="qb", name="qb")
            nc.sync.dma_start(qb[:], qbias[:])
            pb = pp.tile([P, 6], f32, tag="pb", name="pb")
            nc.sync.dma_start(pb[:], pbias[:])
            vbr = pp.tile([1, C], f32, tag="vbr", name="vbr")
            nc.sync.dma_start(vbr[:], vbias[:])
            vb = pp.tile([P, C], f32, tag="vb", name="vb")
            nc.gpsimd.partition_broadcast(vb[:, :], vbr[0:1, :])
            xtall = pp.tile([P, 6, NT], bf16, tag="xtall", name="xtall")
            wtall = pp.tile([P, 6, 3 * C], bf16, tag="wtall", name="wtall")
            ptall = pp.tile([P, 6, C], bf16, tag="ptall", name="ptall")
            xt = [xtall[:, i, :] for i in range(6)]
            wt = [wtall[:, i, :] for i in range(6)]
            pt = [ptall[:, i, :] for i in range(6)]
            # strided bulk loads, ordered by first use: batch-0 x + V-block
            # weight columns (the b0 V projection starts the kernel), then
            # batch-1 x, then the Q/K weight columns
            nc.sync.dma_start(
                xtall[:, :, 0:N], xT[:, 0:N].rearrange("(i p) n -> p i n", p=P)
            )
            nc.sync.dma_start(
                wtall[:, :, 2 * C : 3 * C],
                wqkvT[:, 2 * C : 3 * C].rearrange("(i p) n -> p i n", p=P),
            )
            nc.sync.dma_start(
                xtall[:, :, N:NT], xT[:, N:NT].rearrange("(i p) n -> p i n", p=P)
            )
            nc.sync.dma_start(
                wtall[:, :, 0 : 2 * C],
                wqkvT[:, 0 : 2 * C].rearrange("(i p) n -> p i n", p=P),
            )

            # ---------------- persistent result tiles ----------------------
            # qk[t] for t in 0..11: [128, NT] bf16, outc block t (q: 0-5, k: 6-11)
            qk = []
            for t in range(12):
                qk.append(pp.tile([P, NT], bf16, tag=f"qk{t}", name=f"qk{t}"))
            # o[t]: [128, NT] bf16 -- O^T assembled for the projection
            o = []
            for t in range(6):
                o.append(pp.tile([P, NT], bf16, tag=f"o{t}", name=f"o{t}"))
            v = [[None] * 5 for _ in range(BPC)]

            def qk_group(t):
                # Q^T/K^T projection for outc block t; Q evicts on DVE
                # (tensor_scalar mul+bias), K evicts on GpSimd (plain copy)
                for (f0, fsz) in TFREE:
                    ps = ps_s.tile([P, 1024], f32, tag="ps_s", name="psmm")
                    for ki in range(6):
                        nc.tensor.matmul(
                            ps[:, 0:fsz],
                            wt[ki][:, P * t : P * (t + 1)],
                            xt[ki][:, f0 : f0 + fsz],
                            start=(ki == 0),
                            stop=(ki == 5),
                        )
                    if t < 6:  # q: scale 1/8 + bias (pre-scaled on host)
                        nc.scalar.activation(
                            qk[t][:, f0 : f0 + fsz],
                            ps[:, 0:fsz],
                            Act.Identity,
                            bias=qb[:, t : t + 1],
                            scale=0.125,
                        )
                    else:  # k: plain copy (k bias is zero)
                        nc.scalar.copy(qk[t][:, f0 : f0 + fsz], ps[:, 0:fsz])

            def v_group(b, j):
                # V projection (natural layout) for batch b, token tile j
                # v[b][j]: [nksz, 780] bf16, 12 head-blocks of [1 | V_h(64)]
                nk0, nksz = NKT[j]
                vt = pp.tile([P, 12 * 65], bf16, tag=f"v{b}_{j}", name=f"v{b}_{j}")
                v[b][j] = vt
                v3 = vt[:, :].rearrange("p (h w) -> p h w", w=65)
                nc.gpsimd.memset(v3[:, :, 64:65], 1.0)
                tok0 = b * N + nk0
                for half in range(2):  # outc halves of 384 = 6 heads
                    f0 = 384 * half
                    ps = ps_s.tile([P, 1024], f32, tag="ps_s", name="psmm")
                    for ki in range(6):
                        nc.tensor.matmul(
                            ps[0:nksz, 0:384],
                            xt[ki][:, tok0 : tok0 + nksz],
                            wt[ki][:, 2 * C + f0 : 2 * C + f0 + 384],
                            start=(ki == 0),
                            stop=(ki == 5),
                        )
                    ps3 = ps[0:nksz, 0:384].rearrange("p (h w) -> p h w", w=64)
                    vb3 = vb[0:nksz, f0 : f0 + 384].rearrange(
                        "p (h w) -> p h w", w=64
                    )
                    nc.vector.tensor_tensor(
                        v3[0:nksz, 6 * half : 6 * half + 6, 0:64],
                        ps3[:, :, :],
                        vb3[:, :, :],
                        op=Alu.add,
                    )

            def proj_group(t, f0, fsz, eng):
                ps = ps_s.tile([P, 1024], f32, tag="ps_s", name="psmm")
                for ki in range(6):
                    nc.tensor.matmul(
                        ps[:, 0:fsz],
                        pt[ki][:, P * t : P * (t + 1)],
                        o[ki][:, f0 : f0 + fsz],
                        start=(ki == 0),
                        stop=(ki == 5),
                    )
                ot = oevp.tile([P, 512], f32, tag="oev", name="oev")
                if eng == "act":
                    nc.scalar.activation(
                        ot[:, 0:fsz], ps[:, 0:fsz], Act.Identity,
                        bias=pb[:, t : t + 1],
                    )
                else:
                    nc.vector.tensor_scalar(
                        ot[:, 0:fsz], ps[:, 0:fsz], pb[:, t : t + 1], None,
                        op0=Alu.add,
                    )
                nc.sync.dma_start(out[P * t : P * (t + 1), f0 : f0 + fsz], ot[:, 0:fsz])

            # -------------- pipelined attention phases ---------------------
            # staP/rbaP: [128, 2*5N] bf16, head parity pr at cols [pr*5N, (pr+1)*5N)
            def s_prologue(b, h0):
                # rel-bias DMAs + tile allocs for the pair (h0, h0+1)
                rbaP = relp.tile([P, 10 * N], bf16, tag="rba", name="rba")
                staP = stp.tile([P, 10 * N], bf16, tag="sta", name="sta")
                for hh in (h0, h0 + 1):
                    pr = hh % 2
                    r3 = rbaP[:, 5 * N * pr : 5 * N * pr + 5 * N].rearrange(
                        "p (j q) -> p j q", q=N
                    )
                    nc.sync.dma_start(
                        r3[:, :, :],
                        relbT[hh, 0:640, :].rearrange("(j p) q -> p j q", p=P),
                    )
                rump = ps_r.tile([P, 1024], f32, tag="rump", name="rump")
                return staP, rbaP, rump

            def s_step(b, h0, st, j):
                # S matmuls + one pair-merged exp evict for nk tile j
                staP, rbaP, rump = st
                qt = h0 // 2
                nk0, nksz = NKT[j]
                ps = ps_s.tile([P, 1024], f32, tag="ps_s", name="pss")
                for hh in (h0, h0 + 1):
                    pr = hh % 2
                    qoff = pr * 64
                    lk = qk[6 + qt][qoff : qoff + 64, b * N + nk0 : b * N + nk0 + nksz]
                    nc.tensor.matmul(
                        ps[0:nksz, 512 * pr : 512 * pr + 512],
                        lk,
                        qk[qt][qoff : qoff + 64, b * N : b * N + 512],
                        start=True,
                        stop=True,
                    )
                    nc.tensor.matmul(
                        rump[0:nksz, 512 * pr + 65 * j : 512 * pr + 65 * j + 65],
                        lk,
                        qk[qt][qoff : qoff + 64, b * N + 512 : b * N + N],
                        start=True,
                        stop=True,
                    )
                # one exp for both heads' 512-chunks (adjacent psum banks)
                s2 = staP[:, :].rearrange("p (h q) -> p h q", h=2)
                p2 = ps[:, :].rearrange("p (h q) -> p h q", h=2)
                nc.scalar.activation(
                    s2[0:nksz, :, N * j : N * j + 512],
                    p2[0:nksz, :, :],
                    Act.Exp,
                )

            def s_epilogue(st):
                # one strided exp for all ten 65-wide rumps of the pair
                # (rows 65:128 of the j=4 chunks hold garbage -- never read)
                staP, rbaP, rump = st
                s3 = staP[:, :].rearrange("p (h j q) -> p h j q", h=2, q=N)
                r3 = rump[:, :].rearrange("p (h q) -> p h q", h=2)[
                    :, :, 0:325
                ].rearrange("p h (j q) -> p h j q", q=65)
                nc.scalar.activation(
                    s3[:, :, 0:5, 512:577], r3[:, :, :, :], Act.Exp
                )

            def mult_phase(st):
                # multiplicative rel-bias, both heads at once; first 3 nk
                # blocks on DVE, last 2 on GpSimd (engine balance)
                staP, rbaP, rump = st
                s2 = staP[:, :].rearrange("p (h q) -> p h q", h=2)
                r2 = rbaP[:, :].rearrange("p (h q) -> p h q", h=2)
                nc.vector.tensor_tensor(
                    s2[:, :, 0 : 3 * N], s2[:, :, 0 : 3 * N], r2[:, :, 0 : 3 * N],
                    op=Alu.mult,
                )
                nc.vector.tensor_tensor(
                    s2[:, :, 3 * N : 5 * N], s2[:, :, 3 * N : 5 * N],
                    r2[:, :, 3 * N : 5 * N],
                    op=Alu.mult,
                )

            def av_alloc(hh):
                # pass-1 psum: [65, 512] (one bank per head, both heads live)
                ost = dnp.tile([64, N], bf16, tag="ost", name="ost")
                ps1 = ps_o.tile([65, 512], f32, tag="o", name="pso1")
                return ost, ps1

            def av_step(b, hh, av, staP, j):
                ost, ps1 = av
                pr = hh % 2
                nk0, nksz = NKT[j]
                lv = v[b][j][0:nksz, 65 * hh : 65 * hh + 65]
                nc.tensor.matmul(
                    ps1[0:65, 0:512],
                    lv,
                    staP[0:nksz, 5 * N * pr + N * j : 5 * N * pr + N * j + 512],
                    start=(j == 0),
                    stop=(j == 4),
                )

            def av_evict1(av):
                # denominator row straight out of PSUM (parallel with the
                # O eviction): dr on ACT (small), O rows on DVE
                ost, ps1 = av
                dr = dnp.tile([1, N], f32, tag="dr", name="dr")
                nc.vector.tensor_copy(dr[0:1, 0:512], ps1[64:65, 0:512])
                nc.vector.tensor_copy(ost[0:64, 0:512], ps1[0:64, 0:512])
                return dr

            def av_pass2(b, h0, avs, drs, staP):
                # nq rump (cols 512:577) for both heads, one ps_r-pool tile
                # (bank-split h0/h1); then finish: evict + recip + broadcast
                ps2 = ps_r.tile([P, 1024], f32, tag="rump", name="pso2")
                fins = {}
                for hh in (h0, h0 + 1):
                    pr = hh % 2
                    for j, (nk0, nksz) in enumerate(NKT):
                        lv = v[b][j][0:nksz, 65 * hh : 65 * hh + 65]
                        nc.tensor.matmul(
                            ps2[0:65, 512 * pr : 512 * pr + 65],
                            lv,
                            staP[0:nksz, 5 * N * pr + N * j + 512 : 5 * N * pr + N * j + N],
                            start=(j == 0),
                            stop=(j == 4),
                        )
                for hh in (h0, h0 + 1):
                    pr = hh % 2
                    ost = avs[hh][0]
                    dr = drs[hh]
                    nc.vector.tensor_copy(dr[0:1, 512:577], ps2[64:65, 512 * pr : 512 * pr + 65])
                    nc.vector.tensor_copy(
                        ost[0:64, 512:577], ps2[0:64, 512 * pr : 512 * pr + 65]
                    )
                    rr = dnp.tile([1, N], f32, tag="rr", name="rr")
                    nc.vector.reciprocal_approx_fast(rr[0:1, 0:N], dr[0:1, 0:N])
                    rb = dnp.tile([64, N], f32, tag="rbb", name="rbb")
                    nc.gpsimd.partition_broadcast(rb[0:64, 0:N], rr[0:1, 0:N])
                    fins[hh] = (ost, rb)
                return fins

            def omult(b, hh, fin):
                ost, rb = fin
                qt = hh // 2
                qoff = (hh % 2) * 64
                nc.vector.tensor_tensor(
                    o[qt][qoff : qoff + 64, b * N : b * N + N],
                    ost[0:64, 0:N],
                    rb[0:64, 0:N],
                    op=Alu.mult,
                )

            # proj token chunks, batch-aligned
            PFREE0 = [(0, 512), (512, 65)]           # batch 0 tokens
            PFREE1 = [(577, 512), (1089, 65)]        # batch 1 tokens

            # ------------- pipelined emission schedule ---------------------
            for j in range(5):
                v_group(0, j)
            qk_group(0)
            qk_group(6)
            qk_group(1)
            qk_group(7)

            pairs = [(b, h0) for b in range(BPC) for h0 in range(0, 12, 2)]
            # lead-in: pair 0's S runs without an AV partner
            st = s_prologue(*pairs[0])
            for j in range(5):
                s_step(*pairs[0], st, j)
            s_epilogue(st)
            mult_phase(st)
            cur = {0: st}
            for k in range(12):
                b, h0 = pairs[k]
                staP = cur[k][0]
                avs = {hh: av_alloc(hh) for hh in (h0, h0 + 1)}
                if k + 1 < 12:
                    nb, nh0 = pairs[k + 1]
                    cur[k + 1] = s_prologue(nb, nh0)
                    # interleave: S of pair k+1 with AV of pair k, per nk tile
                    for j in range(5):
                        s_step(nb, nh0, cur[k + 1], j)
                        for hh in (h0, h0 + 1):
                            av_step(b, hh, avs[hh], staP, j)
                    s_epilogue(cur[k + 1])
                else:
                    for j in range(5):
                        for hh in (h0, h0 + 1):
                            av_step(b, hh, avs[hh], staP, j)
                drs = {hh: av_evict1(avs[hh]) for hh in (h0, h0 + 1)}
                if k + 1 < 12:
                    mult_phase(cur[k + 1])
                fins = av_pass2(b, h0, avs, drs, staP)
                for hh in (h0, h0 + 1):
                    omult(b, hh, fins[hh])
                del cur[k]
                # fillers: dense PE work with no dependence on pair k+1
                if k <= 3:                       # Q/K projections, 2 pairs ahead
                    qk_group(k + 2)
                    qk_group(k + 8)
                elif k == 4:
                    nc.sync.dma_start(
                        ptall[:, :, :], projT[:, :].rearrange("(i p) n -> p i n", p=P)
                    )
                    v_group(1, 0)
                    v_group(1, 1)
                elif k == 5:
                    v_group(1, 2)
                    v_group(1, 3)
                    v_group(1, 4)
                elif 7 <= k <= 8:                # b0 proj, one outc tile each
                    t = k - 7
                    for (f0, fsz) in PFREE0:
                        proj_group(t, f0, fsz, "act")
            # tail: held-back b0 proj fills the last normalize wait
            for t in (2, 3, 4, 5):
                for (f0, fsz) in PFREE0:
                    proj_group(t, f0, fsz, "act")
            # ----- remaining output projection: merged per-chunk DMAs ------
            for ci, (f0, fsz) in enumerate(PFREE1):
                otall = pp.tile([P, 6, fsz], f32, tag=f"oevall{ci}", name="oevall")
                for t in range(6):
                    ps = ps_s.tile([P, 1024], f32, tag="ps_s", name="psmm")
                    for ki in range(6):
                        nc.tensor.matmul(
                            ps[:, 0:fsz],
                            pt[ki][:, P * t : P * (t + 1)],
                            o[ki][:, f0 : f0 + fsz],
                            start=(ki == 0),
                            stop=(ki == 5),
                        )
                    nc.scalar.activation(
                        otall[:, t, 0:fsz], ps[:, 0:fsz], Act.Identity,
                        bias=pb[:, t : t + 1],
                    )
                nc.sync.dma_start(
                    out[:, f0 : f0 + fsz].rearrange("(t p) n -> p t n", p=P),
                    otall[:, :, 0:fsz],
                )

    nc.compile()
    return nc


def _get_nc():
    if "nc" not in _CACHE:
        _CACHE["nc"] = _build()
    return _CACHE["nc"]


def make_in_maps(x, rel_pos_bias, qkv_w, q_bias, v_bias, proj_w, proj_b):
    bf = ml_dtypes.bfloat16
    x = np.asarray(x, dtype=np.float32)
    rel_pos_bias = np.asarray(rel_pos_bias, dtype=np.float32)
    qkv_w = np.asarray(qkv_w, dtype=np.float32)
    q_bias = np.asarray(q_bias, dtype=np.float32)
    v_bias = np.asarray(v_bias, dtype=np.float32)
    proj_w = np.asarray(proj_w, dtype=np.float32)
    proj_b = np.asarray(proj_b, dtype=np.float32)

    wqkvT = np.ascontiguousarray(qkv_w.T).astype(bf)                    # [768, 2304]
    qbias = np.ascontiguousarray((q_bias * 0.125).reshape(6, P).T)      # [128, 6]
    vbias = np.ascontiguousarray(v_bias[None, :])                       # [1, 768]
    # exp of the transposed rel-pos bias: applied multiplicatively after exp(S);
    # rows padded 577->640 (5*128) so each head loads as a single strided DMA
    relbT = np.zeros((H, 640, N), dtype=bf)
    relbT[:, :N, :] = np.exp(rel_pos_bias[0].transpose(0, 2, 1)).astype(bf)
    projT = np.ascontiguousarray(proj_w.T).astype(bf)                   # [768, 768]
    pbias = np.ascontiguousarray(proj_b.reshape(6, P).T)                # [128, 6]

    in_maps = []
    for c in range(NCORES):
        xT = np.ascontiguousarray(
            x[BPC * c : BPC * (c + 1)].reshape(NT, C).T
        ).astype(bf)                                                    # [768, 1154]
        in_maps.append(
            dict(
                xT=xT,
                wqkvT=wqkvT,
                qbias=qbias,
                vbias=vbias,
                relbT=relbT,
                projT=projT,
                pbias=pbias,
            )
        )
    return in_maps


def kernel(x, rel_pos_bias, qkv_w, q_bias, v_bias, proj_w, proj_b):
    from concourse import bass_utils

    in_maps = make_in_maps(x, rel_pos_bias, qkv_w, q_bias, v_bias, proj_w, proj_b)
    nc = _get_nc()
    res = bass_utils.run_bass_kernel_spmd(nc, in_maps, core_ids=list(range(NCORES)))
    outs = []
    for c in range(NCORES):
        oT = res.results[c]["out"]                                      # [768, 1154]
        outs.append(np.ascontiguousarray(oT.T).reshape(BPC, N, C))
    return np.concatenate(outs, axis=0)
